# revision 1
# baseline (speedup 1.0000x reference)
"""GAT (4-layer, 2-head) message-passing kernel for 8 TRN2 NeuronCores.

Strategy (dst-sharded, node-major K-slot layout):
  - Nodes sharded by dst range (12500/core). Per core, nodes are sorted by
    in-degree and grouped into chunks of 128; chunk j has a static in-edge
    slot capacity caps[j] (max degree over cores in that chunk position),
    identical across cores so one SPMD program serves all 8.
  - Per layer: dense phase computes r-rows [xl_h0 |1| xl_h1 |1| a_src a_dst]
    for local nodes (one matmul per 128 nodes), AllGather replicates the full
    node table, then per chunk: caps[j] single-offset indirect gathers fetch
    the k-th in-edge's source row for all 128 nodes (one row per partition —
    the HW-supported indirect-DMA form), per-edge softmax weights
    w = exp(leaky_relu(a_src[src]+a_dst[dst])) are computed with the dst term
    broadcast per partition (dst == the partition's node), messages are
    scaled by w, and the segment softmax reduces along the free (k) axis:
    out = sum_k(w*xl) / sum_k(w).  Pad slots point at a poisoned table row
    (xl=0, a_src=-60) so they contribute ~0.
"""

import math
import os
import numpy as np

# ---------------------------------------------------------------- problem dims
N_NODES = 100000
N_EDGES = 1600000
N_CORES = 8
DIM_IN = 128
HEADS = 2
HID = 64
DIM_OUT = 32

PART = 128        # nodes per chunk / SBUF partitions


class Cfg:
    def __init__(self, n_nodes=N_NODES, n_edges=N_EDGES, n_cores=N_CORES):
        assert n_nodes % n_cores == 0
        self.n_nodes = n_nodes
        self.n_edges = n_edges
        self.n_cores = n_cores
        self.shard = n_nodes // n_cores
        self.nchunk = math.ceil(self.shard / PART)
        self.nrow = self.nchunk * PART + 1      # +1 pad/poison row
        self.nrows_all = self.nrow * n_cores
        self.caps = None
        # per-layer geometry: HW = head group width (xl + const col)
        self.HW = [65, 65, 65, 33]
        self.RW = [136, 136, 136, 72]   # r-row width (2*HW + as(2) + ad(2) + pad)
        self.GW = [132, 132, 132, 68]   # gathered prefix (2*HW + as(2))
        self.OW = [128, 128, 128, 32]   # out width per layer

    def finalize_caps(self, caps):
        self.caps = [int(c) for c in caps]
        self.nblk = sum(self.caps)
        self.blk_start = np.concatenate([[0], np.cumsum(self.caps)]).astype(np.int64)


# ------------------------------------------------------------------ host prep
def preprocess(cfg: Cfg, edge_index: np.ndarray):
    src = np.asarray(edge_index[0], dtype=np.int64)
    dst = np.asarray(edge_index[1], dtype=np.int64)
    C, SH, NC = cfg.n_cores, cfg.shard, cfg.nchunk

    owner = dst // SH
    deg = np.zeros((C, SH), dtype=np.int64)
    for c in range(C):
        deg[c] = np.bincount(dst[owner == c] - c * SH, minlength=SH)

    # per-core: sort nodes by degree desc -> chunk j = nodes[j*128:(j+1)*128]
    slot_node = np.full((C, cfg.nrow), -1, dtype=np.int64)
    node_slot = np.full(cfg.n_nodes, -1, dtype=np.int64)   # slot within core
    maxdeg = np.zeros((C, NC), dtype=np.int64)
    for c in range(C):
        order = np.argsort(-deg[c], kind="stable")
        ns = min(SH, NC * PART)
        slot_node[c, :ns] = order[:ns] + c * SH
        node_slot[order + c * SH] = np.arange(SH)
        d_sorted = deg[c][order]
        pad = np.zeros(NC * PART - SH, dtype=np.int64)
        d_pad = np.concatenate([d_sorted, pad])
        maxdeg[c] = d_pad.reshape(NC, PART).max(axis=1)
    caps = np.maximum(maxdeg.max(axis=0), 1)
    cfg.finalize_caps(caps)

    # global table row of node n (within its core's region)
    node_row = np.full(cfg.n_nodes, -1, dtype=np.int64)
    for c in range(C):
        rows = slot_node[c]
        real = rows >= 0
        node_row[rows[real]] = c * cfg.nrow + np.nonzero(real)[0]
    assert (node_row >= 0).all()
    pad_row = np.array([c * cfg.nrow + cfg.nrow - 1 for c in range(C)])

    # place edges: edge -> (core, slot=node_slot[dst], k=arrival index)
    NBLK = cfg.nblk
    srcg = np.zeros((C, PART, NBLK), dtype=np.int64)
    for c in range(C):
        srcg[c, :, :] = pad_row[c]
    ds_order = np.argsort(dst, kind="stable")
    ds = dst[ds_order]
    uniq, first = np.unique(ds, return_index=True)
    cnt = np.diff(np.concatenate([first, [len(ds)]]))
    k_of = np.arange(len(ds)) - np.repeat(first, cnt)   # arrival index per dst
    e_sl = node_slot[ds]
    e_ch = e_sl // PART
    e_pa = e_sl % PART
    e_ow = ds // SH
    col = cfg.blk_start[e_ch] + k_of
    srcg[e_ow, e_pa, col] = node_row[src[ds_order]]

    return dict(srcg=srcg.astype(np.int32), slot_node=slot_node,
                node_row=node_row)


def pack_weights(cfg, W, asrc, adst, layer):
    """Wcat [128, RW]: [W_h0 | 0 | W_h1 | 0 | W@As | W@Ad | 0pad]."""
    HW, RW = cfg.HW[layer], cfg.RW[layer]
    dout = HW - 1
    Wcat = np.zeros((W.shape[0], RW), dtype=np.float32)
    for h in range(2):
        Wcat[:, h * HW:h * HW + dout] = W[:, h * dout:(h + 1) * dout]
        Wcat[:, 2 * HW + h] = W[:, h * dout:(h + 1) * dout] @ asrc[h]
        Wcat[:, 2 * HW + 2 + h] = W[:, h * dout:(h + 1) * dout] @ adst[h]
    return Wcat


# ---------------------------------------------------------------- numpy model
def emulate_numpy(cfg, prep, inputs, stash=None):
    C = cfg.n_cores
    x = np.asarray(inputs["x"], np.float32)
    params = [(pack_weights(cfg, np.asarray(inputs[f"W{l}"], np.float32),
                            np.asarray(inputs[f"asrc{l}"], np.float32),
                            np.asarray(inputs[f"adst{l}"], np.float32), l),
               np.asarray(inputs[f"b{l}"], np.float32)) for l in range(4)]
    srcg = prep["srcg"]
    slot_node = prep["slot_node"]

    h = np.zeros((C, cfg.nrow, DIM_IN), np.float32)
    for c in range(C):
        real = slot_node[c] >= 0
        h[c][real] = x[slot_node[c][real]]

    out_final = np.zeros((C, cfg.nrow - 1, DIM_OUT), np.float32)
    for l in range(4):
        Wcat, b = params[l]
        HW, RW, GW = cfg.HW[l], cfg.RW[l], cfg.GW[l]
        r = np.einsum("cnk,kr->cnr", h, Wcat)
        r[:, :, HW - 1] = 1.0
        r[:, :, 2 * HW - 1] = 1.0
        # poison pad row
        r[:, -1, :] = 0.0
        r[:, -1, 2 * HW:2 * HW + 2] = -60.0
        if stash is not None:
            stash[f"r{l}"] = r.copy()
        rfull = r.reshape(C * cfg.nrow, RW)
        for c in range(C):
            for j in range(cfg.nchunk):
                K = cfg.caps[j]
                cols = slice(cfg.blk_start[j], cfg.blk_start[j] + K)
                G = rfull[srcg[c][:, cols]]          # [128, K, RW]
                ad = r[c, j * PART:(j + 1) * PART, 2 * HW + 2:2 * HW + 4]
                z = G[:, :, 2 * HW:2 * HW + 2] + ad[:, None, :]
                zlr = np.where(z > 0, z, 0.2 * z)
                w = np.exp(zlr)
                Gs = G[:, :, :2 * HW].reshape(PART, K, 2, HW).copy()
                Gs *= w[:, :, :, None]
                red = Gs.sum(axis=1).reshape(PART, 2 * HW)   # [128, 130]
                den = red[:, [HW - 1, 2 * HW - 1]] + 1e-16
                o = np.stack([red[:, hh * HW:hh * HW + HW - 1] / den[:, hh:hh + 1]
                              for hh in range(2)], axis=1)
                if l < 3:
                    hn = o.reshape(PART, -1) + b[None, :]
                    h[c, j * PART:(j + 1) * PART] = np.maximum(hn, 0.0)
                else:
                    m = o.mean(axis=1) + b[None, :]
                    mx = m.max(axis=1, keepdims=True)
                    lse = mx + np.log(np.exp(m - mx).sum(axis=1, keepdims=True))
                    out_final[c, j * PART:(j + 1) * PART] = m - lse
        h[:, -1, :] = 0.0
    res = np.zeros((cfg.n_nodes, DIM_OUT), np.float32)
    for c in range(C):
        sn = slot_node[c][:-1]
        real = sn >= 0
        res[sn[real]] = out_final[c][real]
    return res


# ---------------------------------------------------------------- bass kernel
def build_nc(cfg: Cfg):
    import concourse.bass as bass
    import concourse.mybir as mybir
    import concourse.tile as tile
    from concourse import bacc
    from concourse.masks import make_identity

    f32 = mybir.dt.float32
    i32 = mybir.dt.int32
    AF = mybir.ActivationFunctionType
    OP = mybir.AluOpType

    nc = bacc.Bacc(num_devices=cfg.n_cores)
    NBLK, NCH = cfg.nblk, cfg.nchunk

    x_in = nc.declare_dram_parameter("x", [cfg.nrow, DIM_IN], f32, False)
    srcg_in = nc.declare_dram_parameter("srcg", [PART, NBLK], i32, False)
    wcat_in = [nc.declare_dram_parameter(f"wcat{l}", [DIM_IN, cfg.RW[l]], f32, False)
               for l in range(4)]
    bias_in = [nc.declare_dram_parameter(f"bias{l}", [PART, cfg.OW[l]], f32, False)
               for l in range(4)]
    poison_in = nc.declare_dram_parameter("poison", [1, 136], f32, False)
    out_p = nc.declare_dram_parameter("out", [cfg.nrow - 1, DIM_OUT], f32, True)

    h_int = nc.dram_tensor("h_int", [cfg.nrow, DIM_IN], f32)
    r_shard = nc.dram_tensor("r_shard", [cfg.nrow, 136], f32)
    r_shard3 = nc.dram_tensor("r_shard3", [cfg.nrow, 72], f32)
    aspc = "Shared" if cfg.n_cores > 4 else "Local"
    rfA = nc.dram_tensor("rfA", [cfg.nrows_all, 136], f32, addr_space=aspc)
    rfB = nc.dram_tensor("rfB", [cfg.nrows_all, 136], f32, addr_space=aspc)
    rf3 = nc.dram_tensor("rf3", [cfg.nrows_all, 72], f32, addr_space=aspc)

    rg = [list(range(cfg.n_cores))]

    with tile.TileContext(nc) as tc:
        with tc.tile_pool(name="const", bufs=1) as cp, \
             tc.tile_pool(name="gbuf", bufs=2) as gp, \
             tc.tile_pool(name="small", bufs=3) as mp, \
             tc.tile_pool(name="dense", bufs=3) as dp, \
             tc.tile_pool(name="psd", bufs=2, space="PSUM") as ppd:

            srcg = cp.tile([PART, NBLK], i32)
            nc.sync.dma_start(out=srcg[:], in_=srcg_in[:])
            wct, bst = [], []
            for l in range(4):
                t = cp.tile([DIM_IN, cfg.RW[l]], f32, tag=f"wc{l}")
                nc.sync.dma_start(out=t[:], in_=wcat_in[l][:])
                wct.append(t)
                t2 = cp.tile([PART, cfg.OW[l]], f32, tag=f"bs{l}")
                nc.sync.dma_start(out=t2[:], in_=bias_in[l][:])
                bst.append(t2)
            ident = cp.tile([PART, PART], f32)
            make_identity(nc, ident[:])

            for l in range(4):
                HW, RW, GW = cfg.HW[l], cfg.RW[l], cfg.GW[l]
                h_src = x_in if l == 0 else h_int
                rsh = r_shard if l < 3 else r_shard3
                rfull = (rfA, rfB, rfA, rf3)[l]

                # ---------------- dense phase ----------------
                for g in range(NCH):
                    ht = dp.tile([PART, DIM_IN], f32, tag="h")
                    nc.sync.dma_start(out=ht[:],
                                      in_=h_src[g * PART:(g + 1) * PART, :])
                    pt = ppd.tile([PART, PART], f32, tag="ht")
                    nc.tensor.transpose(out=pt[:], in_=ht[:], identity=ident[:])
                    hts = dp.tile([PART, PART], f32, tag="hts")
                    nc.vector.tensor_copy(out=hts[:], in_=pt[:])
                    pr = ppd.tile([PART, RW], f32, tag="pr")
                    nc.tensor.matmul(out=pr[:], lhsT=hts[:], rhs=wct[l][:],
                                     start=True, stop=True)
                    rt = dp.tile([PART, RW], f32, tag="rt")
                    nc.vector.tensor_copy(out=rt[:], in_=pr[:])
                    nc.vector.memset(rt[:, HW - 1:HW], 1.0)
                    nc.vector.memset(rt[:, 2 * HW - 1:2 * HW], 1.0)
                    nc.sync.dma_start(out=rsh[g * PART:(g + 1) * PART, :],
                                      in_=rt[:])
                # poison the pad row (xl=0, as=-60)
                nc.sync.dma_start(out=rsh[cfg.nrow - 1:cfg.nrow, 0:RW],
                                  in_=poison_in[0:1, 0:RW])

                # ---------------- all-gather ----------------
                nc.gpsimd.collective_compute(
                    "AllGather", mybir.AluOpType.bypass, replica_groups=rg,
                    ins=[rsh[:]], outs=[rfull[:]])

                # ---------------- edge phase ----------------
                for g in range(NCH):
                    K = cfg.caps[g]
                    b0 = int(cfg.blk_start[g])

                    gt = gp.tile([PART, K * GW], f32, tag="g")
                    gv = gt[:].rearrange("p (k r) -> p k r", r=GW)
                    for k in range(K):
                        nc.gpsimd.indirect_dma_start(
                            out=gv[:, k, :], out_offset=None, in_=rfull[:],
                            in_offset=bass.IndirectOffsetOnAxis(
                                ap=srcg[:, b0 + k:b0 + k + 1], axis=0))
                    adt = mp.tile([PART, 2], f32, tag="ad")
                    nc.sync.dma_start(
                        out=adt[:],
                        in_=rsh[g * PART:(g + 1) * PART, 2 * HW + 2:2 * HW + 4])
                    zt = mp.tile([PART, K * 2], f32, tag="z")
                    nc.vector.tensor_tensor(
                        out=zt[:].rearrange("p (k t) -> p k t", t=2),
                        in0=gv[:, :, 2 * HW:2 * HW + 2],
                        in1=adt[:].unsqueeze(1).to_broadcast([PART, K, 2]),
                        op=OP.add)
                    lt = mp.tile([PART, K * 2], f32, tag="lr")
                    nc.vector.tensor_scalar_mul(out=lt[:], in0=zt[:],
                                                scalar1=0.2)
                    nc.vector.tensor_tensor(out=lt[:], in0=lt[:], in1=zt[:],
                                            op=OP.max)
                    wt = mp.tile([PART, K * 2], f32, tag="w")
                    nc.scalar.activation(out=wt[:], in_=lt[:], func=AF.Exp)

                    ghead = gv[:, :, 0:2 * HW].rearrange(
                        "p k (h c) -> p k h c", c=HW)
                    wb = wt[:].rearrange("p (k h) -> p k h", h=2) \
                        .unsqueeze(3).to_broadcast([PART, K, 2, HW])
                    nc.vector.tensor_tensor(out=ghead, in0=ghead, in1=wb,
                                            op=OP.mult)

                    red = mp.tile([PART, 2 * HW], f32, tag="red")
                    nc.vector.tensor_reduce(
                        out=red[:],
                        in_=gt[:].rearrange("p (k r) -> p r k", r=GW)
                        [:, 0:2 * HW, :],
                        axis=mybir.AxisListType.X, op=OP.add)

                    dt2 = mp.tile([PART, 2], f32, tag="den")
                    rv = red[:].rearrange("p (h c) -> p h c", c=HW)
                    nc.vector.tensor_scalar(
                        out=dt2[:], in0=rv[:, :, HW - 1:HW].squeeze(2),
                        scalar1=1e-16, scalar2=None, op0=OP.add)
                    rc = mp.tile([PART, 2], f32, tag="rcp")
                    nc.vector.reciprocal(out=rc[:], in_=dt2[:])

                    if l < 3:
                        ot = dp.tile([PART, 2 * (HW - 1)], f32, tag="o")
                        nc.vector.tensor_tensor(
                            out=ot[:].rearrange("p (h c) -> p h c", c=HW - 1),
                            in0=rv[:, :, 0:HW - 1],
                            in1=rc[:].unsqueeze(2)
                                .to_broadcast([PART, 2, HW - 1]),
                            op=OP.mult)
                        nc.vector.tensor_tensor(out=ot[:], in0=ot[:],
                                                in1=bst[l][:], op=OP.add)
                        nc.vector.tensor_scalar_max(out=ot[:], in0=ot[:],
                                                    scalar1=0.0)
                        nc.sync.dma_start(
                            out=h_int[g * PART:(g + 1) * PART, :], in_=ot[:])
                    else:
                        oh = mp.tile([PART, 2 * (HW - 1)], f32, tag="oh")
                        nc.vector.tensor_tensor(
                            out=oh[:].rearrange("p (h c) -> p h c", c=HW - 1),
                            in0=rv[:, :, 0:HW - 1],
                            in1=rc[:].unsqueeze(2)
                                .to_broadcast([PART, 2, HW - 1]),
                            op=OP.mult)
                        m1 = mp.tile([PART, HW - 1], f32, tag="m1")
                        ohv = oh[:].rearrange("p (h c) -> p h c", c=HW - 1)
                        nc.vector.tensor_tensor(out=m1[:], in0=ohv[:, 0, :],
                                                in1=ohv[:, 1, :], op=OP.add)
                        nc.vector.tensor_scalar_mul(out=m1[:], in0=m1[:],
                                                    scalar1=0.5)
                        nc.vector.tensor_tensor(out=m1[:], in0=m1[:],
                                                in1=bst[l][:], op=OP.add)
                        mx = mp.tile([PART, 1], f32, tag="mx")
                        nc.vector.tensor_reduce(out=mx[:], in_=m1[:],
                                                axis=mybir.AxisListType.X,
                                                op=OP.max)
                        sh = mp.tile([PART, HW - 1], f32, tag="sh")
                        nc.vector.tensor_scalar(out=sh[:], in0=m1[:],
                                                scalar1=mx[:], scalar2=None,
                                                op0=OP.subtract)
                        ex = mp.tile([PART, HW - 1], f32, tag="ex")
                        nc.scalar.activation(out=ex[:], in_=sh[:], func=AF.Exp)
                        sm = mp.tile([PART, 1], f32, tag="sm")
                        nc.vector.tensor_reduce(out=sm[:], in_=ex[:],
                                                axis=mybir.AxisListType.X,
                                                op=OP.add)
                        ln = mp.tile([PART, 1], f32, tag="ln")
                        nc.scalar.activation(out=ln[:], in_=sm[:], func=AF.Ln)
                        fin = mp.tile([PART, DIM_OUT], f32, tag="fin")
                        nc.vector.tensor_scalar(out=fin[:], in0=sh[:],
                                                scalar1=ln[:], scalar2=None,
                                                op0=OP.subtract)
                        nc.sync.dma_start(
                            out=out_p[g * PART:(g + 1) * PART, :], in_=fin[:])
    return nc


def make_in_maps(cfg, prep, inputs):
    x = np.asarray(inputs["x"], np.float32)
    in_maps = []
    for c in range(cfg.n_cores):
        sn = prep["slot_node"][c]
        real = sn >= 0
        xs = np.zeros((cfg.nrow, DIM_IN), np.float32)
        xs[real] = x[sn[real]]
        poison = np.zeros((1, 136), np.float32)
        poison[0, 130:132] = -60.0
        m = dict(x=xs, srcg=prep["srcg"][c], poison=poison)
        for l in range(4):
            m[f"wcat{l}"] = pack_weights(
                cfg, np.asarray(inputs[f"W{l}"], np.float32),
                np.asarray(inputs[f"asrc{l}"], np.float32),
                np.asarray(inputs[f"adst{l}"], np.float32), l)
            b = np.asarray(inputs[f"b{l}"], np.float32)
            m[f"bias{l}"] = np.broadcast_to(b[None, :], (PART, cfg.OW[l])).copy()
        in_maps.append(m)
    return in_maps


# ---------------------------------------------------------------- entry point
def kernel(**inputs) -> np.ndarray:
    cfg = Cfg()
    edge_index = np.asarray(inputs["edge_index"])
    prep = preprocess(cfg, edge_index)
    nc = build_nc(cfg)
    nc.finalize()
    in_maps = make_in_maps(cfg, prep, inputs)

    from concourse.bass_utils import run_bass_kernel_spmd
    res = run_bass_kernel_spmd(nc, in_maps, list(range(cfg.n_cores)),
                               trace=bool(int(os.environ.get("GAT_TRACE", "0"))))
    if res.exec_time_ns is not None:
        print(f"HW exec time: {res.exec_time_ns} ns")
    out = np.zeros((cfg.n_nodes, DIM_OUT), np.float32)
    for c in range(cfg.n_cores):
        sn = prep["slot_node"][c][:-1]
        real = sn >= 0
        out[sn[real]] = res.results[c]["out"][real]
    return out



# revision 7
# speedup vs baseline: 1.0053x; 1.0053x over previous
"""GAT (4-layer, 2-head) message-passing kernel for 8 TRN2 NeuronCores.

Strategy (dst-sharded, node-major K-slot layout):
  - Nodes sharded by dst range (12500/core). Per core, nodes are sorted by
    in-degree and grouped into chunks of 128; chunk j has a static in-edge
    slot capacity caps[j] (max degree over cores in that chunk position),
    identical across cores so one SPMD program serves all 8.
  - Per layer: dense phase computes r-rows [xl_h0 |1| xl_h1 |1| a_src a_dst]
    for local nodes (one matmul per 128 nodes), AllGather replicates the full
    node table, then per chunk: caps[j] single-offset indirect gathers fetch
    the k-th in-edge's source row for all 128 nodes (one row per partition —
    the HW-supported indirect-DMA form), per-edge softmax weights
    w = exp(leaky_relu(a_src[src]+a_dst[dst])) are computed with the dst term
    broadcast per partition (dst == the partition's node), messages are
    scaled by w, and the segment softmax reduces along the free (k) axis:
    out = sum_k(w*xl) / sum_k(w).  Pad slots point at a poisoned table row
    (xl=0, a_src=-60) so they contribute ~0.
"""

import math
import os
import numpy as np

VARIANT_NO_AG = bool(int(os.environ.get("GAT_NO_AG", "0")))
VARIANT_DENSE_GATHER = bool(int(os.environ.get("GAT_DENSE_GATHER", "0")))

# ---------------------------------------------------------------- problem dims
N_NODES = 100000
N_EDGES = 1600000
N_CORES = 8
DIM_IN = 128
HEADS = 2
HID = 64
DIM_OUT = 32

PART = 128        # nodes per chunk / SBUF partitions


class Cfg:
    def __init__(self, n_nodes=N_NODES, n_edges=N_EDGES, n_cores=N_CORES):
        assert n_nodes % n_cores == 0
        self.n_nodes = n_nodes
        self.n_edges = n_edges
        self.n_cores = n_cores
        self.shard = n_nodes // n_cores
        self.nchunk = math.ceil(self.shard / PART)
        self.nrow = self.nchunk * PART + 1      # +1 pad/poison row
        self.nrows_all = self.nrow * n_cores
        self.caps = None
        # per-layer geometry: HW = head group width (xl + const col)
        self.HW = [65, 65, 65, 33]
        self.RW = [136, 136, 136, 72]   # r-row width (2*HW + as(2) + ad(2) + pad)
        self.GW = [132, 132, 132, 68]   # gathered prefix (2*HW + as(2))
        self.OW = [128, 128, 128, 32]   # out width per layer

    def finalize_caps(self, caps):
        self.caps = [int(c) for c in caps]
        self.nblk = sum(self.caps)
        self.blk_start = np.concatenate([[0], np.cumsum(self.caps)]).astype(np.int64)


# ------------------------------------------------------------------ host prep
def preprocess(cfg: Cfg, edge_index: np.ndarray):
    src = np.asarray(edge_index[0], dtype=np.int64)
    dst = np.asarray(edge_index[1], dtype=np.int64)
    C, SH, NC = cfg.n_cores, cfg.shard, cfg.nchunk

    owner = dst // SH
    deg = np.zeros((C, SH), dtype=np.int64)
    for c in range(C):
        deg[c] = np.bincount(dst[owner == c] - c * SH, minlength=SH)

    # per-core: sort nodes by degree desc -> chunk j = nodes[j*128:(j+1)*128]
    slot_node = np.full((C, cfg.nrow), -1, dtype=np.int64)
    node_slot = np.full(cfg.n_nodes, -1, dtype=np.int64)   # slot within core
    maxdeg = np.zeros((C, NC), dtype=np.int64)
    for c in range(C):
        order = np.argsort(-deg[c], kind="stable")
        ns = min(SH, NC * PART)
        slot_node[c, :ns] = order[:ns] + c * SH
        node_slot[order + c * SH] = np.arange(SH)
        d_sorted = deg[c][order]
        pad = np.zeros(NC * PART - SH, dtype=np.int64)
        d_pad = np.concatenate([d_sorted, pad])
        maxdeg[c] = d_pad.reshape(NC, PART).max(axis=1)
    caps = np.maximum(maxdeg.max(axis=0), 1)
    cfg.finalize_caps(caps)

    # global table row of node n (within its core's region)
    node_row = np.full(cfg.n_nodes, -1, dtype=np.int64)
    for c in range(C):
        rows = slot_node[c]
        real = rows >= 0
        node_row[rows[real]] = c * cfg.nrow + np.nonzero(real)[0]
    assert (node_row >= 0).all()
    pad_row = np.array([c * cfg.nrow + cfg.nrow - 1 for c in range(C)])

    # place edges: edge -> (core, slot=node_slot[dst], k=arrival index)
    NBLK = cfg.nblk
    srcg = np.zeros((C, PART, NBLK), dtype=np.int64)
    for c in range(C):
        srcg[c, :, :] = pad_row[c]
    ds_order = np.argsort(dst, kind="stable")
    ds = dst[ds_order]
    uniq, first = np.unique(ds, return_index=True)
    cnt = np.diff(np.concatenate([first, [len(ds)]]))
    k_of = np.arange(len(ds)) - np.repeat(first, cnt)   # arrival index per dst
    e_sl = node_slot[ds]
    e_ch = e_sl // PART
    e_pa = e_sl % PART
    e_ow = ds // SH
    col = cfg.blk_start[e_ch] + k_of
    srcg[e_ow, e_pa, col] = node_row[src[ds_order]]

    return dict(srcg=srcg.astype(np.int32), slot_node=slot_node,
                node_row=node_row)


def pack_weights(cfg, W, asrc, adst, layer):
    """Wcat [128, RW]: [W_h0 | 0 | W_h1 | 0 | W@As | W@Ad | 0pad]."""
    HW, RW = cfg.HW[layer], cfg.RW[layer]
    dout = HW - 1
    Wcat = np.zeros((W.shape[0], RW), dtype=np.float32)
    for h in range(2):
        Wcat[:, h * HW:h * HW + dout] = W[:, h * dout:(h + 1) * dout]
        Wcat[:, 2 * HW + h] = W[:, h * dout:(h + 1) * dout] @ asrc[h]
        Wcat[:, 2 * HW + 2 + h] = W[:, h * dout:(h + 1) * dout] @ adst[h]
    return Wcat


# ---------------------------------------------------------------- numpy model
def emulate_numpy(cfg, prep, inputs, stash=None):
    C = cfg.n_cores
    x = np.asarray(inputs["x"], np.float32)
    params = [(pack_weights(cfg, np.asarray(inputs[f"W{l}"], np.float32),
                            np.asarray(inputs[f"asrc{l}"], np.float32),
                            np.asarray(inputs[f"adst{l}"], np.float32), l),
               np.asarray(inputs[f"b{l}"], np.float32)) for l in range(4)]
    srcg = prep["srcg"]
    slot_node = prep["slot_node"]

    h = np.zeros((C, cfg.nrow, DIM_IN), np.float32)
    for c in range(C):
        real = slot_node[c] >= 0
        h[c][real] = x[slot_node[c][real]]

    out_final = np.zeros((C, cfg.nrow - 1, DIM_OUT), np.float32)
    for l in range(4):
        Wcat, b = params[l]
        HW, RW, GW = cfg.HW[l], cfg.RW[l], cfg.GW[l]
        r = np.einsum("cnk,kr->cnr", h, Wcat)
        r[:, :, HW - 1] = 1.0
        r[:, :, 2 * HW - 1] = 1.0
        # poison pad row
        r[:, -1, :] = 0.0
        r[:, -1, 2 * HW:2 * HW + 2] = -60.0
        if stash is not None:
            stash[f"r{l}"] = r.copy()
        rfull = r.reshape(C * cfg.nrow, RW)
        for c in range(C):
            for j in range(cfg.nchunk):
                K = cfg.caps[j]
                cols = slice(cfg.blk_start[j], cfg.blk_start[j] + K)
                G = rfull[srcg[c][:, cols]]          # [128, K, RW]
                ad = r[c, j * PART:(j + 1) * PART, 2 * HW + 2:2 * HW + 4]
                z = G[:, :, 2 * HW:2 * HW + 2] + ad[:, None, :]
                zlr = np.where(z > 0, z, 0.2 * z)
                w = np.exp(zlr)
                Gs = G[:, :, :2 * HW].reshape(PART, K, 2, HW).copy()
                Gs *= w[:, :, :, None]
                red = Gs.sum(axis=1).reshape(PART, 2 * HW)   # [128, 130]
                den = red[:, [HW - 1, 2 * HW - 1]] + 1e-16
                o = np.stack([red[:, hh * HW:hh * HW + HW - 1] / den[:, hh:hh + 1]
                              for hh in range(2)], axis=1)
                if l < 3:
                    hn = o.reshape(PART, -1) + b[None, :]
                    h[c, j * PART:(j + 1) * PART] = np.maximum(hn, 0.0)
                else:
                    m = o.mean(axis=1) + b[None, :]
                    mx = m.max(axis=1, keepdims=True)
                    lse = mx + np.log(np.exp(m - mx).sum(axis=1, keepdims=True))
                    out_final[c, j * PART:(j + 1) * PART] = m - lse
        h[:, -1, :] = 0.0
    res = np.zeros((cfg.n_nodes, DIM_OUT), np.float32)
    for c in range(C):
        sn = slot_node[c][:-1]
        real = sn >= 0
        res[sn[real]] = out_final[c][real]
    return res


# ---------------------------------------------------------------- bass kernel
def build_nc(cfg: Cfg):
    import concourse.bass as bass
    import concourse.mybir as mybir
    import concourse.tile as tile
    from concourse import bacc
    from concourse.masks import make_identity

    f32 = mybir.dt.float32
    i32 = mybir.dt.int32
    AF = mybir.ActivationFunctionType
    OP = mybir.AluOpType

    nc = bacc.Bacc(num_devices=cfg.n_cores)
    NBLK, NCH = cfg.nblk, cfg.nchunk

    x_in = nc.declare_dram_parameter("x", [cfg.nrow, DIM_IN], f32, False)
    srcg_in = nc.declare_dram_parameter("srcg", [PART, NBLK], i32, False)
    wcat_in = [nc.declare_dram_parameter(f"wcat{l}", [DIM_IN, cfg.RW[l]], f32, False)
               for l in range(4)]
    bias_in = [nc.declare_dram_parameter(f"bias{l}", [PART, cfg.OW[l]], f32, False)
               for l in range(4)]
    poison_in = nc.declare_dram_parameter("poison", [1, 136], f32, False)
    out_p = nc.declare_dram_parameter("out", [cfg.nrow - 1, DIM_OUT], f32, True)

    h_int = nc.dram_tensor("h_int", [cfg.nrow, DIM_IN], f32)
    r_shard = nc.dram_tensor("r_shard", [cfg.nrow, 136], f32)
    r_shard3 = nc.dram_tensor("r_shard3", [cfg.nrow, 72], f32)
    aspc = "Shared" if cfg.n_cores > 4 else "Local"
    rfA = nc.dram_tensor("rfA", [cfg.nrows_all, 136], f32, addr_space=aspc)
    rfB = nc.dram_tensor("rfB", [cfg.nrows_all, 136], f32, addr_space=aspc)
    rf3 = nc.dram_tensor("rf3", [cfg.nrows_all, 72], f32, addr_space=aspc)

    rg = [list(range(cfg.n_cores))]

    with tile.TileContext(nc) as tc:
        with tc.tile_pool(name="const", bufs=1) as cp, \
             tc.tile_pool(name="gbuf", bufs=2) as gp, \
             tc.tile_pool(name="small", bufs=3) as mp, \
             tc.tile_pool(name="dense", bufs=3) as dp, \
             tc.tile_pool(name="psd", bufs=2, space="PSUM") as ppd:

            srcg = cp.tile([PART, NBLK], i32)
            nc.sync.dma_start(out=srcg[:], in_=srcg_in[:])
            wct, bst = [], []
            for l in range(4):
                t = cp.tile([DIM_IN, cfg.RW[l]], f32, tag=f"wc{l}")
                nc.sync.dma_start(out=t[:], in_=wcat_in[l][:])
                wct.append(t)
                t2 = cp.tile([PART, cfg.OW[l]], f32, tag=f"bs{l}")
                nc.sync.dma_start(out=t2[:], in_=bias_in[l][:])
                bst.append(t2)
            ident = cp.tile([PART, PART], f32)
            make_identity(nc, ident[:])

            for l in range(4):
                HW, RW, GW = cfg.HW[l], cfg.RW[l], cfg.GW[l]
                h_src = x_in if l == 0 else h_int
                rsh = r_shard if l < 3 else r_shard3
                rfull = (rfA, rfB, rfA, rf3)[l]

                # ---------------- dense phase ----------------
                for g in range(NCH):
                    ht = dp.tile([PART, DIM_IN], f32, tag="h")
                    nc.sync.dma_start(out=ht[:],
                                      in_=h_src[g * PART:(g + 1) * PART, :])
                    pt = ppd.tile([PART, PART], f32, tag="ht")
                    nc.tensor.transpose(out=pt[:], in_=ht[:], identity=ident[:])
                    hts = dp.tile([PART, PART], f32, tag="hts")
                    nc.vector.tensor_copy(out=hts[:], in_=pt[:])
                    pr = ppd.tile([PART, RW], f32, tag="pr")
                    nc.tensor.matmul(out=pr[:], lhsT=hts[:], rhs=wct[l][:],
                                     start=True, stop=True)
                    rt = dp.tile([PART, RW], f32, tag="rt")
                    nc.vector.tensor_copy(out=rt[:], in_=pr[:])
                    nc.vector.memset(rt[:, HW - 1:HW], 1.0)
                    nc.vector.memset(rt[:, 2 * HW - 1:2 * HW], 1.0)
                    nc.sync.dma_start(out=rsh[g * PART:(g + 1) * PART, :],
                                      in_=rt[:])
                # poison the pad row (xl=0, as=-60)
                nc.sync.dma_start(out=rsh[cfg.nrow - 1:cfg.nrow, 0:RW],
                                  in_=poison_in[0:1, 0:RW])

                # ---------------- all-gather ----------------
                if VARIANT_NO_AG:
                    # timing-only: local copy into own shard slot
                    nc.sync.dma_start(
                        out=rfull[0:cfg.nrow, :], in_=rsh[:])
                else:
                    nc.gpsimd.collective_compute(
                        "AllGather", mybir.AluOpType.bypass, replica_groups=rg,
                        ins=[rsh[:]], outs=[rfull[:]])

                # ---------------- edge phase ----------------
                for g in range(NCH):
                    K = cfg.caps[g]
                    b0 = int(cfg.blk_start[g])

                    gt = gp.tile([PART, K * GW], f32, tag="g")
                    gv = gt[:].rearrange("p (k r) -> p k r", r=GW)
                    if VARIANT_DENSE_GATHER:
                        # timing-only: same bytes, contiguous reads
                        nc.sync.dma_start(
                            out=gv[:],
                            in_=rfull[0:K * PART, 0:GW]
                            .rearrange("(k p) r -> p k r", p=PART))
                    else:
                        for k in range(K):
                            nc.gpsimd.indirect_dma_start(
                                out=gv[:, k, :], out_offset=None, in_=rfull[:],
                                in_offset=bass.IndirectOffsetOnAxis(
                                    ap=srcg[:, b0 + k:b0 + k + 1], axis=0))
                    adt = mp.tile([PART, 2], f32, tag="ad")
                    nc.sync.dma_start(
                        out=adt[:],
                        in_=rsh[g * PART:(g + 1) * PART, 2 * HW + 2:2 * HW + 4])
                    zt = mp.tile([PART, K * 2], f32, tag="z")
                    nc.vector.tensor_tensor(
                        out=zt[:].rearrange("p (k t) -> p k t", t=2),
                        in0=gv[:, :, 2 * HW:2 * HW + 2],
                        in1=adt[:].unsqueeze(1).to_broadcast([PART, K, 2]),
                        op=OP.add)
                    lt = mp.tile([PART, K * 2], f32, tag="lr")
                    nc.vector.tensor_scalar_mul(out=lt[:], in0=zt[:],
                                                scalar1=0.2)
                    nc.vector.tensor_tensor(out=lt[:], in0=lt[:], in1=zt[:],
                                            op=OP.max)
                    wt = mp.tile([PART, K * 2], f32, tag="w")
                    nc.scalar.activation(out=wt[:], in_=lt[:], func=AF.Exp)

                    ghead = gv[:, :, 0:2 * HW].rearrange(
                        "p k (h c) -> p k h c", c=HW)
                    wb = wt[:].rearrange("p (k h) -> p k h", h=2) \
                        .unsqueeze(3).to_broadcast([PART, K, 2, HW])
                    nc.vector.tensor_tensor(out=ghead, in0=ghead, in1=wb,
                                            op=OP.mult)

                    red = mp.tile([PART, 2 * HW], f32, tag="red")
                    nc.vector.tensor_reduce(
                        out=red[:],
                        in_=gt[:].rearrange("p (k r) -> p r k", r=GW)
                        [:, 0:2 * HW, :],
                        axis=mybir.AxisListType.X, op=OP.add)

                    dt2 = mp.tile([PART, 2], f32, tag="den")
                    rv = red[:].rearrange("p (h c) -> p h c", c=HW)
                    nc.vector.tensor_scalar(
                        out=dt2[:], in0=rv[:, :, HW - 1:HW].squeeze(2),
                        scalar1=1e-16, scalar2=None, op0=OP.add)
                    rc = mp.tile([PART, 2], f32, tag="rcp")
                    nc.vector.reciprocal(out=rc[:], in_=dt2[:])

                    if l < 3:
                        ot = dp.tile([PART, 2 * (HW - 1)], f32, tag="o")
                        nc.vector.tensor_tensor(
                            out=ot[:].rearrange("p (h c) -> p h c", c=HW - 1),
                            in0=rv[:, :, 0:HW - 1],
                            in1=rc[:].unsqueeze(2)
                                .to_broadcast([PART, 2, HW - 1]),
                            op=OP.mult)
                        nc.vector.tensor_tensor(out=ot[:], in0=ot[:],
                                                in1=bst[l][:], op=OP.add)
                        nc.vector.tensor_scalar_max(out=ot[:], in0=ot[:],
                                                    scalar1=0.0)
                        nc.sync.dma_start(
                            out=h_int[g * PART:(g + 1) * PART, :], in_=ot[:])
                    else:
                        oh = mp.tile([PART, 2 * (HW - 1)], f32, tag="oh")
                        nc.vector.tensor_tensor(
                            out=oh[:].rearrange("p (h c) -> p h c", c=HW - 1),
                            in0=rv[:, :, 0:HW - 1],
                            in1=rc[:].unsqueeze(2)
                                .to_broadcast([PART, 2, HW - 1]),
                            op=OP.mult)
                        m1 = mp.tile([PART, HW - 1], f32, tag="m1")
                        ohv = oh[:].rearrange("p (h c) -> p h c", c=HW - 1)
                        nc.vector.tensor_tensor(out=m1[:], in0=ohv[:, 0, :],
                                                in1=ohv[:, 1, :], op=OP.add)
                        nc.vector.tensor_scalar_mul(out=m1[:], in0=m1[:],
                                                    scalar1=0.5)
                        nc.vector.tensor_tensor(out=m1[:], in0=m1[:],
                                                in1=bst[l][:], op=OP.add)
                        mx = mp.tile([PART, 1], f32, tag="mx")
                        nc.vector.tensor_reduce(out=mx[:], in_=m1[:],
                                                axis=mybir.AxisListType.X,
                                                op=OP.max)
                        sh = mp.tile([PART, HW - 1], f32, tag="sh")
                        nc.vector.tensor_scalar(out=sh[:], in0=m1[:],
                                                scalar1=mx[:], scalar2=None,
                                                op0=OP.subtract)
                        ex = mp.tile([PART, HW - 1], f32, tag="ex")
                        nc.scalar.activation(out=ex[:], in_=sh[:], func=AF.Exp)
                        sm = mp.tile([PART, 1], f32, tag="sm")
                        nc.vector.tensor_reduce(out=sm[:], in_=ex[:],
                                                axis=mybir.AxisListType.X,
                                                op=OP.add)
                        ln = mp.tile([PART, 1], f32, tag="ln")
                        nc.scalar.activation(out=ln[:], in_=sm[:], func=AF.Ln)
                        fin = mp.tile([PART, DIM_OUT], f32, tag="fin")
                        nc.vector.tensor_scalar(out=fin[:], in0=sh[:],
                                                scalar1=ln[:], scalar2=None,
                                                op0=OP.subtract)
                        nc.sync.dma_start(
                            out=out_p[g * PART:(g + 1) * PART, :], in_=fin[:])
    return nc


def make_in_maps(cfg, prep, inputs):
    x = np.asarray(inputs["x"], np.float32)
    in_maps = []
    for c in range(cfg.n_cores):
        sn = prep["slot_node"][c]
        real = sn >= 0
        xs = np.zeros((cfg.nrow, DIM_IN), np.float32)
        xs[real] = x[sn[real]]
        poison = np.zeros((1, 136), np.float32)
        poison[0, 130:132] = -60.0
        m = dict(x=xs, srcg=prep["srcg"][c], poison=poison)
        for l in range(4):
            m[f"wcat{l}"] = pack_weights(
                cfg, np.asarray(inputs[f"W{l}"], np.float32),
                np.asarray(inputs[f"asrc{l}"], np.float32),
                np.asarray(inputs[f"adst{l}"], np.float32), l)
            b = np.asarray(inputs[f"b{l}"], np.float32)
            m[f"bias{l}"] = np.broadcast_to(b[None, :], (PART, cfg.OW[l])).copy()
        in_maps.append(m)
    return in_maps


# ---------------------------------------------------------------- entry point
def kernel(**inputs) -> np.ndarray:
    cfg = Cfg()
    edge_index = np.asarray(inputs["edge_index"])
    prep = preprocess(cfg, edge_index)
    nc = build_nc(cfg)
    nc.finalize()
    in_maps = make_in_maps(cfg, prep, inputs)

    from concourse.bass_utils import run_bass_kernel_spmd
    res = run_bass_kernel_spmd(nc, in_maps, list(range(cfg.n_cores)),
                               trace=bool(int(os.environ.get("GAT_TRACE", "0"))))
    if res.exec_time_ns is not None:
        print(f"HW exec time: {res.exec_time_ns} ns")
    out = np.zeros((cfg.n_nodes, DIM_OUT), np.float32)
    for c in range(cfg.n_cores):
        sn = prep["slot_node"][c][:-1]
        real = sn >= 0
        out[sn[real]] = res.results[c]["out"][real]
    return out

